# revision 11
# baseline (speedup 1.0000x reference)
"""BitNet ternary linear layer on 8 Trainium2 NeuronCores.

y = x @ (W * s)^T with x (32, 4096) f32, W (11008, 4096) ternary {-1,0,+1}.

Strategy (memory-bound — the kernel is a W-stream at HBM line rate):
  - Tensor-parallel: shard W rows (out_features) across 8 cores, 1376 each;
    x replicated; per-core [32, 1376] outputs concatenated on the host.
  - Host-side prep (free): fold s into x, transpose to PE layouts, store W
    as fp8 E4M3 (ternary is EXACT in fp8). x is split into NSPLIT=2 fp8
    planes stacked along the stationary M dim (~2^-8 x precision; measured
    rel err ~7e-4 vs the 2e-2 gate).
  - fp8 DoubleRow matmuls: K=256 per pass, 16 passes accumulate into
    per-chunk PSUM tiles (512/512/352 output columns per chunk).
  - W streams on both HWDGE queues in pass order, greedily balanced by
    bytes. Early stripes are single passes (fine arrival granularity keeps
    the cold PE dense — each queue delivers serially, so coarse early
    stripes starve the PE, reset the HAM clock gate, and stall the 8
    recycled DMA sem lanes). Middle stripes are 2 passes for SDMA
    efficiency. The final pass is 3 chunk-column slices so each chunk's
    closing matmul + PSUM drain + output DMA pipelines as its slice lands.
  - x rides the sync queue first (xa: passes 0-7) and mid-stream (xb).
  - A short warmup matmul burst (memset tile, N=256) keeps the PE busy
    from kernel start so HAM reaches K=8/8 (2.4 GHz) early.
  - Outputs leave as fp16 raw planes (cast during the PSUM->SBUF copy);
    the scaled plane-sum runs on the host in f32.
"""

import numpy as np
import ml_dtypes

N_CORES = 8
B, I, O = 32, 4096, 11008
OC = O // N_CORES        # 1376
NP = I // 256            # 16 DoubleRow passes (K=256 each)
NSPLIT = 2               # fp8 planes of x
ALPHA = 16.0             # residual plane q scaled by ALPHA**q
M = NSPLIT * B           # 64 stationary columns
OCHUNKS = [(0, 512), (512, 512), (1024, 352)]
# W stripes over passes 1-14: single passes alternating queues. The
# consumer-visible arrival of a stripe is its DMA completion semaphore,
# which fires per-stripe and lags the data by ~1.5-2us — coarse stripes
# create multi-us arrival gaps that idle the PE and re-throttle the HAM
# clock gate (measured: a 2-pass-stripe schedule collapsed the whole MM
# stream to 1.2 GHz). Single-pass stripes keep arrivals at ~0.7-0.9us,
# below even the cold-PE pass time, so the PE never goes idle.
# Passes 0 and 15 are further split into 3 chunk-column slices: pass 0
# so the first matmul starts as early as possible, pass 15 so each
# chunk's closing matmul + PSUM drain + output DMA pipelines per-slice.
STRIPES = [
    (1, 1, "sync"),
    (2, 1, "scalar"),
    (3, 1, "sync"),
    (4, 1, "scalar"),
    (5, 2, "sync"),
    (7, 2, "scalar"),
    (9, 2, "sync"),
    (11, 2, "scalar"),
    (13, 1, "sync"),
    (14, 1, "scalar"),
]
SLICE_Q = ("scalar", "sync", "scalar")  # queues for pass-0/15 chunk slices
LAST = NP - 1            # pass 15
WARMUP_MMS = 10

_BUILT = None


def _build():
    import concourse.bacc as bacc
    import concourse.mybir as mybir
    from concourse.tile import TileContext

    f8 = mybir.dt.float8e4
    f16 = mybir.dt.float16
    nc = bacc.Bacc("TRN2", target_bir_lowering=False, debug=False)
    xt = nc.dram_tensor("xt", (128, NP * 2 * M), f8, kind="ExternalInput")
    # wt layout per partition: [p0..p14 blocks of (2, OC)] then pass 15
    # reordered chunk-major: [c0 (2,512), c1 (2,512), c2 (2,352)].
    wt = nc.dram_tensor("wt", (128, NP * 2 * OC), f8, kind="ExternalInput")
    # raw per-plane partials in fp16; the scaled plane-sum happens on host
    yp = nc.dram_tensor("yp", (M, OC), f16, kind="ExternalOutput")

    with TileContext(nc) as tc:
        with (
            tc.tile_pool(name="xp", bufs=1) as xp,
            tc.tile_pool(name="wp", bufs=1) as wp,
            tc.tile_pool(name="pp", bufs=1, space="PSUM") as pp,
            tc.tile_pool(name="op", bufs=1) as op,
        ):
            # x leads on sync (phase-offsets the two rings); pass-0 slices
            # lead on scalar so the first matmul starts as early as possible.
            xs = xp.tile([128, NP * 2 * M], f8, name="xs", tag="xs")
            nc.sync.dma_start(xs[:, :], xt[:, :])
            x4 = xs[:, :].rearrange("p (j i m) -> p j i m", j=NP, i=2, m=M)

            def chunk_slices(jpass, prefix):
                # 3 chunk-column slice tiles of one pass (host lays the
                # pass out chunk-major: [c0 (2,512), c1 (2,512), c2 (2,352)])
                base = jpass * 2 * OC
                tiles = []
                cum = 0
                for c, (o0, n) in enumerate(OCHUNKS):
                    t = wp.tile([128, 2 * n], f8, name=f"{prefix}c{c}",
                                tag=f"{prefix}c{c}")
                    eng = nc.scalar if SLICE_Q[c] == "scalar" else nc.sync
                    eng.dma_start(t[:, :], wt[:, base + cum : base + cum + 2 * n])
                    tiles.append(t)
                    cum += 2 * n
                return tiles

            w0 = chunk_slices(0, "w0")

            stripes = {}
            for s, (p0, np_s, q) in enumerate(STRIPES):
                w = wp.tile([128, np_s * 2 * OC], f8, name=f"w{s}", tag=f"w{s}")
                o0 = p0 * 2 * OC
                eng = nc.scalar if q == "scalar" else nc.sync
                eng.dma_start(w[:, :], wt[:, o0 : o0 + np_s * 2 * OC])
                stripes[p0] = (np_s, w)

            w15 = chunk_slices(LAST, "w15")

            # PE warmup on a memset tile (vector engine is idle early):
            # a continuous burst bridging kernel start to the first real
            # matmul (~3.8us) so the HAM activity window sees no idle gap
            # and flips to 2.4 GHz right as real data arrives.
            wsrc = xp.tile([128, 256], f8, name="wsrc")
            nc.vector.memset(wsrc[:, :], 0.0)
            scratch = pp.tile([128, 256], mybir.dt.float32, name="scratch")
            for wu in range(WARMUP_MMS):
                nc.tensor.matmul(
                    scratch[:, :], wsrc[:, 0:128], wsrc[:, 0:256],
                    start=True, stop=True,
                )

            ps = [
                pp.tile([M, n], mybir.dt.float32, name=f"ps{c}")
                for c, (o0, n) in enumerate(OCHUNKS)
            ]

            def mm(j, c):
                o0, n = OCHUNKS[c]
                if j == 0:
                    w3 = w0[c][:, :].rearrange("p (i o) -> p i o", i=2, o=n)
                elif j == LAST:
                    w3 = w15[c][:, :].rearrange("p (i o) -> p i o", i=2, o=n)
                else:
                    p0 = max(p for p in stripes if p <= j)
                    np_s, w = stripes[p0]
                    w4 = w[:, :].rearrange(
                        "p (jj i o) -> p jj i o", jj=np_s, i=2, o=OC
                    )
                    w3 = w4[:, j - p0, :, o0 : o0 + n]
                nc.tensor.matmul(
                    ps[c][:, :],
                    x4[:, j],
                    w3,
                    start=(j == 0),
                    stop=(j == LAST),
                    perf_mode=mybir.MatmulPerfMode.DoubleRow,
                )

            for j in range(LAST):
                for c in range(len(OCHUNKS)):
                    mm(j, c)
            # close each chunk as its pass-15 slice lands; drain + output
            # DMA overlap the other chunks' closing matmuls.
            dma_eng = (nc.sync, nc.scalar, nc.sync)
            for c, (o0, n) in enumerate(OCHUNKS):
                mm(LAST, c)
                sb = op.tile([M, n], f16, name=f"sb{c}", tag=f"sb{c}")
                if c == 1:
                    nc.scalar.copy(sb[:, :], ps[c][:, :])
                else:
                    nc.vector.tensor_copy(sb[:, :], ps[c][:, :])
                dma_eng[c].dma_start(yp[:, o0 : o0 + n], sb[:, :])

    nc.finalize()
    return nc


def _get_nc():
    global _BUILT
    if _BUILT is None:
        _BUILT = _build()
    return _BUILT


def _fp8_split(v, nsplit):
    """Split v into fp8 planes: v ~= sum_q planes[q] / ALPHA**q."""
    planes = []
    rem = v.astype(np.float32)
    for q in range(nsplit):
        p = (rem * np.float32(ALPHA**q)).astype(ml_dtypes.float8_e4m3fn)
        planes.append(p)
        rem = rem - p.astype(np.float32) / np.float32(ALPHA**q)
    return planes


def _prep_inputs(x, weight, scale_factor):
    x = np.asarray(x, dtype=np.float32)
    weight = np.asarray(weight, dtype=np.float32)
    s = np.float32(np.asarray(scale_factor))

    xsT = (x * s).T.astype(np.float32)                  # [I, B]
    planes = _fp8_split(xsT, NSPLIT)
    stacked = np.concatenate(planes, axis=1)            # [I, M]
    # [I, M] with I = (j, i, p): k = 256j + 128i + p  ->  xt[p, j, i, m]
    xt = np.ascontiguousarray(
        stacked.reshape(NP, 2, 128, M).transpose(2, 0, 1, 3).reshape(128, NP * 2 * M)
    )

    in_maps = []
    for c in range(N_CORES):
        wc = weight[c * OC : (c + 1) * OC, :]           # [OC, I]
        wq = wc.T.astype(ml_dtypes.float8_e4m3fn)       # [I, OC], exact
        # per partition: pass blocks in order; passes 0 and 15 are stored
        # chunk-major inside their block ([c0 (2,512), c1 (2,512), c2 (2,352)])
        wtc = wq.reshape(NP, 2, 128, OC).transpose(2, 0, 1, 3)  # [128, NP, 2, OC]

        def chunk_major(blk):                           # [128, 2, OC]
            return np.concatenate(
                [blk[:, :, o0 : o0 + n].reshape(128, 2 * n) for o0, n in OCHUNKS],
                axis=1,
            )

        segs = [chunk_major(wtc[:, 0])]
        segs.append(wtc[:, 1 : NP - 1].reshape(128, (NP - 2) * 2 * OC))
        segs.append(chunk_major(wtc[:, NP - 1]))
        wtc2 = np.ascontiguousarray(np.concatenate(segs, axis=1))
        in_maps.append({"xt": xt, "wt": wtc2})
    return in_maps


def _run(in_maps, trace=False, tmpdir=None):
    from concourse.bass_utils import run_bass_kernel_spmd

    return run_bass_kernel_spmd(
        _get_nc(), in_maps, core_ids=list(range(N_CORES)), trace=trace, tmpdir=tmpdir
    )


def _combine(yp):
    acc = yp[0:B].astype(np.float32).copy()
    for q in range(1, NSPLIT):
        acc += yp[q * B : (q + 1) * B].astype(np.float32) * np.float32(
            1.0 / ALPHA**q
        )
    return acc


def kernel(x, weight, scale_factor):
    in_maps = _prep_inputs(x, weight, scale_factor)
    try:
        res = _run(in_maps)
    except Exception:
        # transient runtime/device hiccups happen; one retry is cheap and
        # the output is still checked downstream
        res = _run(in_maps)
    return np.concatenate(
        [_combine(res.results[c]["yp"]) for c in range(N_CORES)], axis=1
    )
